# revision 10
# baseline (speedup 1.0000x reference)
"""Trainium2 Bass kernel for nn_BiLinearDotLayer.

Computes, for feature (B,F,E)=(2048,200,64) f32 and weight (F,E,E):
    bilinear[b,i,d] = sum_e feature[b,i,e] * weight[i,e,d]
    out[b,i,j]      = sum_d bilinear[b,i,d] * feature[b,j,d]

Strategy (8 NeuronCores, data-parallel over batch):
  - Each core handles 256 batches; weight replicated.
  - Host pre-transposes feature to featT[e, b, i] and packs even/odd
    batches into SBUF partition halves (p*64+e) so all on-chip tiles use
    128 partitions (full DMA width) and the two batch parities run
    concurrently on the PE array's row-strips (K=64 each).
  - Both einsums run fully on-chip per block of 128 batches; only the
    feature shard and weight are read and only the final (256,200,200)
    f32 output is written per core.
  - Matmuls run in float32r (single-pass fp32, ~1e-4 rel rounding);
    einsum2's moving operand is padded to N=256 for the 1 cycle/row
    fp32r fast path.
"""

import os
import sys

for _p in ("/opt/trn_rl_repo", "/root/.axon_site/_ro/trn_rl_repo"):
    if os.path.isdir(_p) and _p not in sys.path:
        sys.path.insert(0, _p)

import numpy as np

B, F, E = 2048, 200, 64
NCORES = 8
BLOC = B // NCORES            # 256 batches per core
NPAIR = BLOC // 2             # 128 even/odd batch pairs per core
BLOCKS = 2
PPB = NPAIR // BLOCKS         # 64 pairs (128 batches) per block
IGRP = 16                     # einsum1 i's per PSUM group (2 banks)
FPAD = 64                     # ftile column padding so e2 rhs can read N=256

USE_F32R = True               # single-pass fp32r for einsum2 (e1 stays exact fp32:
                              # fp32r requires output base partition 0, which the
                              # parity-1 einsum1 matmuls can't satisfy)

_RUNNER = None


def _build_program():
    import concourse.tile as tile
    from concourse import bacc, mybir

    f32 = mybir.dt.float32
    mmdt = mybir.dt.float32r if USE_F32R else f32
    nc = bacc.Bacc("TRN2", target_bir_lowering=False, debug=False)

    fpk = nc.dram_tensor("fpk", [128, NPAIR * F], mmdt, kind="ExternalInput")
    wpk = nc.dram_tensor("wpk", [128, F * E], f32, kind="ExternalInput")
    out = nc.dram_tensor("out", [BLOC, F, F], f32, kind="ExternalOutput")

    # DRAM view (p, b, ci, j) with i = 2*p + ci: each partition owns two
    # adjacent i-rows so out-DMA descriptor runs are 1600B, not 800B.
    out_v = out.ap().rearrange("b (p ci) j -> p b ci j", ci=2)

    with tile.TileContext(nc) as tc:
        with (
            tc.tile_pool(name="wpool", bufs=3) as wpool,
            tc.tile_pool(name="fpool", bufs=2) as fpool,
            tc.tile_pool(name="bpool", bufs=1) as bpool,
            tc.tile_pool(name="stpool", bufs=2) as stpool,
            tc.tile_pool(name="ps1", bufs=2, space="PSUM") as ps1pool,
            tc.tile_pool(name="ps2", bufs=2, space="PSUM") as ps2pool,
        ):
            cpy = 0  # alternates copies between DVE and ACT
            for k in range(BLOCKS):
                ftile = fpool.tile([128, PPB * F + FPAD], mmdt)
                nc.sync.dma_start(
                    out=ftile[:, : PPB * F],
                    in_=fpk.ap()[:, k * PPB * F : (k + 1) * PPB * F],
                )
                btile = bpool.tile([128, PPB * F], mmdt)

                f3 = ftile[:, : PPB * F].rearrange("p (bb i) -> p bb i", i=F)
                b3w = btile[:].rearrange("p (bb i) -> p i bb", i=F)

                # ---- einsum1: bilinearT[d, bb] per i (both parities) ----
                for i0 in range(0, F, IGRP):
                    gs = min(IGRP, F - i0)
                    wseg = wpool.tile([128, IGRP * E], f32)
                    nc.sync.dma_start(
                        out=wseg[:, : gs * E],
                        in_=wpk.ap()[:, i0 * E : (i0 + gs) * E],
                    )
                    pst = ps1pool.tile([128, IGRP * PPB], f32)
                    for g in range(gs):
                        i = i0 + g
                        for p in (0, 1):
                            nc.tensor.matmul(
                                out=pst[p * 64 : (p + 1) * 64, g * PPB : (g + 1) * PPB],
                                lhsT=wseg[p * 64 : (p + 1) * 64, g * E : (g + 1) * E],
                                rhs=f3[p * 64 : (p + 1) * 64, :, i].bitcast(f32),
                                start=True,
                                stop=True,
                            )
                    src = pst[:, : gs * PPB].rearrange("p (g bb) -> p g bb", bb=PPB)
                    dst = b3w[:, i0 : i0 + gs, :]
                    if cpy % 2 == 0:
                        nc.vector.tensor_copy(out=dst, in_=src)
                    else:
                        nc.scalar.copy(out=dst, in_=src)
                    cpy += 1

                # ---- einsum2: out[b] = bilinear[b] @ feature[b].T ----
                # fp32r fast path wants N>=256: stream 256 rhs columns, keep 200.
                # i-chunks are stride-2 interleaved (ci = i%2) so that output
                # partition p owns DRAM rows i=2p, 2p+1 (contiguous 1600B).
                bt4 = btile[:].rearrange("p (bb i2 ci) -> p bb ci i2", i2=100, ci=2)
                STG = 4  # pairs per staged out-DMA (8 batches, 1.28 MB)
                for m in range(0, PPB, STG):
                    stage = stpool.tile([128, STG * 4 * F], f32)
                    for u in range(STG):
                        bb = m + u
                        ps2 = ps2pool.tile([128, 1024], f32)
                        for ci in (0, 1):
                            for p in (0, 1):
                                nc.tensor.matmul(
                                    out=ps2[
                                        0:100,
                                        p * 512 + ci * 256 : p * 512 + ci * 256 + 256,
                                    ],
                                    lhsT=bt4[p * 64 : (p + 1) * 64, bb, ci, :],
                                    rhs=ftile[
                                        p * 64 : (p + 1) * 64, bb * F : bb * F + 256
                                    ],
                                    start=True,
                                    stop=True,
                                )
                        src = ps2[0:100].rearrange(
                            "q (p ci j) -> q p ci j", p=2, ci=2
                        )[:, :, :, 0:F]
                        dst = stage[0:100, u * 4 * F : (u + 1) * 4 * F].rearrange(
                            "q (p ci j) -> q p ci j", p=2, ci=2
                        )
                        if cpy % 2 == 0:
                            nc.vector.tensor_copy(out=dst, in_=src)
                        else:
                            nc.scalar.copy(out=dst, in_=src)
                        cpy += 1
                    b0 = k * 2 * PPB + 2 * m
                    dma_eng = nc.sync if (m // STG) % 2 == 0 else nc.gpsimd
                    dma_eng.dma_start(
                        out=out_v[:, b0 : b0 + 2 * STG, :, :],
                        in_=stage[0:100, :].rearrange(
                            "p (b ci j) -> p b ci j", ci=2, j=F
                        ),
                    )

    nc.compile()
    return nc


class _Runner:
    """Builds the program once and keeps a reusable sharded jit."""

    def __init__(self):
        self.nc = _build_program()
        import jax
        from jax.sharding import Mesh, PartitionSpec
        from jax.experimental.shard_map import shard_map
        from concourse import mybir
        from concourse import bass2jax

        bass2jax.install_neuronx_cc_hook()
        nc = self.nc

        partition_name = (
            nc.partition_id_tensor.name if nc.partition_id_tensor else None
        )
        in_names, out_names, out_avals, zero_outs = [], [], [], []
        for alloc in nc.m.functions[0].allocations:
            if not isinstance(alloc, mybir.MemoryLocationSet):
                continue
            name = alloc.memorylocations[0].name
            if alloc.kind == "ExternalInput":
                if name != partition_name:
                    in_names.append(name)
            elif alloc.kind == "ExternalOutput":
                shape = tuple(alloc.tensor_shape)
                dtype = mybir.dt.np(alloc.dtype)
                out_names.append(name)
                out_avals.append(jax.core.ShapedArray(shape, dtype))
                zero_outs.append(np.zeros(shape, dtype))
        self.in_names = list(in_names)
        self.out_names = out_names
        self.out_avals = out_avals
        self.zero_outs = zero_outs
        n_params = len(in_names)
        n_outs = len(out_avals)
        in_names_full = in_names + out_names
        if partition_name is not None:
            in_names_full.append(partition_name)
        donate = tuple(range(n_params, n_params + n_outs))

        def _body(*args):
            operands = list(args)
            if partition_name is not None:
                operands.append(bass2jax.partition_id_tensor())
            outs = bass2jax._bass_exec_p.bind(
                *operands,
                out_avals=tuple(out_avals),
                in_names=tuple(in_names_full),
                out_names=tuple(out_names),
                lowering_input_output_aliases=(),
                sim_require_finite=True,
                sim_require_nnan=True,
                nc=nc,
            )
            return tuple(outs)

        devices = jax.devices()[:NCORES]
        mesh = Mesh(np.asarray(devices), ("core",))
        in_specs = (PartitionSpec("core"),) * (n_params + n_outs)
        out_specs = (PartitionSpec("core"),) * n_outs
        self.sharded = jax.jit(
            shard_map(
                _body,
                mesh=mesh,
                in_specs=in_specs,
                out_specs=out_specs,
                check_rep=False,
            ),
            donate_argnums=donate,
            keep_unused=True,
        )

    def run(self, concat_inputs):
        """concat_inputs: dict name -> (8*shape0, ...) array."""
        args = [concat_inputs[n] for n in self.in_names]
        zeros = [
            np.zeros((NCORES * z.shape[0], *z.shape[1:]), z.dtype)
            for z in self.zero_outs
        ]
        outs = self.sharded(*args, *zeros)
        return {n: np.asarray(outs[i]) for i, n in enumerate(self.out_names)}


def _get_runner():
    global _RUNNER
    if _RUNNER is None:
        _RUNNER = _Runner()
    return _RUNNER


def pack_inputs(feature, weight):
    """Host-side packing: returns dict of concatenated per-core inputs."""
    feature = np.ascontiguousarray(np.asarray(feature, dtype=np.float32))
    weight = np.ascontiguousarray(np.asarray(weight, dtype=np.float32))
    # featT pack: fpk[core][p*64+e, bb*F+i] = feature[core*BLOC + 2*bb + p, i, e]
    ft = feature.reshape(NCORES, NPAIR, 2, F, E)  # [core, bb, p, i, e]
    fpk = np.ascontiguousarray(ft.transpose(0, 2, 4, 1, 3)).reshape(
        NCORES * 128, NPAIR * F
    )
    wt = np.ascontiguousarray(weight.transpose(1, 0, 2)).reshape(E, F * E)
    wpk_one = np.concatenate([wt, wt], axis=0)  # (128, F*E)
    wpk = np.tile(wpk_one, (NCORES, 1))
    return {"fpk": fpk, "wpk": wpk}


def kernel(feature, weight):
    r = _get_runner()
    ins = pack_inputs(feature, weight)
    outs = r.run(ins)
    return outs["out"].reshape(B, F, F)


if __name__ == "__main__":
    rng = np.random.default_rng(0)
    feature = rng.standard_normal((B, F, E), dtype=np.float32)
    weight = (0.01 * rng.standard_normal((F, E, E))).astype(np.float32)
    got = kernel(feature, weight)
    bil = np.einsum("bie,ied->bid", feature.astype(np.float64), weight.astype(np.float64))
    ref = np.einsum("bid,bjd->bij", bil, feature.astype(np.float64))
    err = np.abs(got - ref)
    denom = np.abs(ref).max()
    print("max abs err:", err.max(), "rel(scale):", err.max() / denom)
    l2 = np.linalg.norm((got - ref).ravel()) / np.linalg.norm(ref.ravel())
    print("L2 rel:", l2)


# revision 13
# speedup vs baseline: 1.0472x; 1.0472x over previous
"""Trainium2 Bass kernel for nn_BiLinearDotLayer.

Computes, for feature (B,F,E)=(2048,200,64) f32 and weight (F,E,E):
    bilinear[b,i,d] = sum_e feature[b,i,e] * weight[i,e,d]
    out[b,i,j]      = sum_d bilinear[b,i,d] * feature[b,j,d]

Strategy (8 NeuronCores, data-parallel over batch):
  - Each core handles 256 batches; weight replicated.
  - Host pre-transposes feature to featT[e, b, i] and packs even/odd
    batches into SBUF partition halves (p*64+e) so all on-chip tiles use
    128 partitions (full DMA width) and the two batch parities run
    concurrently on the PE array's row-strips (K=64 each).
  - Both einsums run fully on-chip per block of 128 batches; only the
    feature shard and weight are read and only the final (256,200,200)
    f32 output is written per core.
  - Matmuls run in float32r (single-pass fp32, ~1e-4 rel rounding);
    einsum2's moving operand is padded to N=256 for the 1 cycle/row
    fp32r fast path.
"""

import os
import sys

for _p in ("/opt/trn_rl_repo", "/root/.axon_site/_ro/trn_rl_repo"):
    if os.path.isdir(_p) and _p not in sys.path:
        sys.path.insert(0, _p)

import numpy as np

B, F, E = 2048, 200, 64
NCORES = 8
BLOC = B // NCORES            # 256 batches per core
NPAIR = BLOC // 2             # 128 even/odd batch pairs per core
BLOCKS = 2
PPB = NPAIR // BLOCKS         # 64 pairs (128 batches) per block
IGRP = 16                     # einsum1 i's per PSUM group (2 banks)
FPAD = 64                     # ftile column padding so e2 rhs can read N=256

USE_F32R = True               # single-pass fp32r for einsum2 (e1 stays exact fp32:
                              # fp32r requires output base partition 0, which the
                              # parity-1 einsum1 matmuls can't satisfy)

_RUNNER = None


def _build_program():
    import concourse.tile as tile
    from concourse import bacc, mybir

    f32 = mybir.dt.float32
    mmdt = mybir.dt.float32r if USE_F32R else f32
    nc = bacc.Bacc("TRN2", target_bir_lowering=False, debug=False)

    fpk = nc.dram_tensor("fpk", [128, NPAIR * F], mmdt, kind="ExternalInput")
    wpk = nc.dram_tensor("wpk", [128, F * E], f32, kind="ExternalInput")
    # Device-friendly output layout: out_dev[p, b, ci, j] = out[b, 2p+ci, j].
    # Each partition's slice is contiguous in DRAM, so out-DMA descriptors are
    # one 12.8KB run per partition per stage group (vs 800B interleaved runs).
    # The host un-permutes afterwards.
    out = nc.dram_tensor("out", [100, BLOC, 2, F], f32, kind="ExternalOutput")
    out_v = out.ap()

    with tile.TileContext(nc) as tc:
        with (
            tc.tile_pool(name="wpool", bufs=3) as wpool,
            tc.tile_pool(name="fpool", bufs=2) as fpool,
            tc.tile_pool(name="bpool", bufs=1) as bpool,
            tc.tile_pool(name="stpool", bufs=2) as stpool,
            tc.tile_pool(name="ps1", bufs=2, space="PSUM") as ps1pool,
            tc.tile_pool(name="ps2", bufs=2, space="PSUM") as ps2pool,
        ):
            cpy = 0  # alternates copies between DVE and ACT
            for k in range(BLOCKS):
                ftile = fpool.tile([128, PPB * F + FPAD], mmdt)
                nc.sync.dma_start(
                    out=ftile[:, : PPB * F],
                    in_=fpk.ap()[:, k * PPB * F : (k + 1) * PPB * F],
                )
                btile = bpool.tile([128, PPB * F], mmdt)

                f3 = ftile[:, : PPB * F].rearrange("p (bb i) -> p bb i", i=F)
                b3w = btile[:].rearrange("p (bb i) -> p i bb", i=F)

                # ---- einsum1: bilinearT[d, bb] per i (both parities) ----
                for i0 in range(0, F, IGRP):
                    gs = min(IGRP, F - i0)
                    wseg = wpool.tile([128, IGRP * E], f32)
                    nc.sync.dma_start(
                        out=wseg[:, : gs * E],
                        in_=wpk.ap()[:, i0 * E : (i0 + gs) * E],
                    )
                    pst = ps1pool.tile([128, IGRP * PPB], f32)
                    for g in range(gs):
                        i = i0 + g
                        for p in (0, 1):
                            nc.tensor.matmul(
                                out=pst[p * 64 : (p + 1) * 64, g * PPB : (g + 1) * PPB],
                                lhsT=wseg[p * 64 : (p + 1) * 64, g * E : (g + 1) * E],
                                rhs=f3[p * 64 : (p + 1) * 64, :, i].bitcast(f32),
                                start=True,
                                stop=True,
                            )
                    src = pst[:, : gs * PPB].rearrange("p (g bb) -> p g bb", bb=PPB)
                    dst = b3w[:, i0 : i0 + gs, :]
                    if cpy % 2 == 0:
                        nc.vector.tensor_copy(out=dst, in_=src)
                    else:
                        nc.scalar.copy(out=dst, in_=src)
                    cpy += 1

                # ---- einsum2: out[b] = bilinear[b] @ feature[b].T ----
                # fp32r fast path wants N>=256: stream 256 rhs columns, keep 200.
                # i-chunks are stride-2 interleaved (ci = i%2) so that output
                # partition p owns DRAM rows i=2p, 2p+1 (contiguous 1600B).
                bt4 = btile[:].rearrange("p (bb i2 ci) -> p bb ci i2", i2=100, ci=2)
                STG = 4  # pairs per staged out-DMA (8 batches, 1.28 MB)
                for m in range(0, PPB, STG):
                    stage = stpool.tile([128, STG * 4 * F], f32)
                    for u in range(STG):
                        bb = m + u
                        ps2 = ps2pool.tile([128, 1024], f32)
                        for ci in (0, 1):
                            for p in (0, 1):
                                nc.tensor.matmul(
                                    out=ps2[
                                        0:100,
                                        p * 512 + ci * 256 : p * 512 + ci * 256 + 256,
                                    ],
                                    lhsT=bt4[p * 64 : (p + 1) * 64, bb, ci, :],
                                    rhs=ftile[
                                        p * 64 : (p + 1) * 64, bb * F : bb * F + 256
                                    ],
                                    start=True,
                                    stop=True,
                                )
                        src = ps2[0:100].rearrange(
                            "q (p ci j) -> q p ci j", p=2, ci=2
                        )[:, :, :, 0:F]
                        dst = stage[0:100, u * 4 * F : (u + 1) * 4 * F].rearrange(
                            "q (p ci j) -> q p ci j", p=2, ci=2
                        )
                        if cpy % 2 == 0:
                            nc.vector.tensor_copy(out=dst, in_=src)
                        else:
                            nc.scalar.copy(out=dst, in_=src)
                        cpy += 1
                    b0 = k * 2 * PPB + 2 * m
                    dma_eng = nc.sync if (m // STG) % 2 == 0 else nc.scalar
                    dma_eng.dma_start(
                        out=out_v[:, b0 : b0 + 2 * STG, :, :],
                        in_=stage[0:100, :].rearrange(
                            "p (b ci j) -> p b ci j", ci=2, j=F
                        ),
                    )

    nc.compile()
    return nc


class _Runner:
    """Builds the program once and keeps a reusable sharded jit."""

    def __init__(self):
        self.nc = _build_program()
        import jax
        from jax.sharding import Mesh, PartitionSpec
        from jax.experimental.shard_map import shard_map
        from concourse import mybir
        from concourse import bass2jax

        bass2jax.install_neuronx_cc_hook()
        nc = self.nc

        partition_name = (
            nc.partition_id_tensor.name if nc.partition_id_tensor else None
        )
        in_names, out_names, out_avals, zero_outs = [], [], [], []
        for alloc in nc.m.functions[0].allocations:
            if not isinstance(alloc, mybir.MemoryLocationSet):
                continue
            name = alloc.memorylocations[0].name
            if alloc.kind == "ExternalInput":
                if name != partition_name:
                    in_names.append(name)
            elif alloc.kind == "ExternalOutput":
                shape = tuple(alloc.tensor_shape)
                dtype = mybir.dt.np(alloc.dtype)
                out_names.append(name)
                out_avals.append(jax.core.ShapedArray(shape, dtype))
                zero_outs.append(np.zeros(shape, dtype))
        self.in_names = list(in_names)
        self.out_names = out_names
        self.out_avals = out_avals
        self.zero_outs = zero_outs
        n_params = len(in_names)
        n_outs = len(out_avals)
        in_names_full = in_names + out_names
        if partition_name is not None:
            in_names_full.append(partition_name)
        donate = tuple(range(n_params, n_params + n_outs))

        def _body(*args):
            operands = list(args)
            if partition_name is not None:
                operands.append(bass2jax.partition_id_tensor())
            outs = bass2jax._bass_exec_p.bind(
                *operands,
                out_avals=tuple(out_avals),
                in_names=tuple(in_names_full),
                out_names=tuple(out_names),
                lowering_input_output_aliases=(),
                sim_require_finite=True,
                sim_require_nnan=True,
                nc=nc,
            )
            return tuple(outs)

        devices = jax.devices()[:NCORES]
        mesh = Mesh(np.asarray(devices), ("core",))
        in_specs = (PartitionSpec("core"),) * (n_params + n_outs)
        out_specs = (PartitionSpec("core"),) * n_outs
        self.sharded = jax.jit(
            shard_map(
                _body,
                mesh=mesh,
                in_specs=in_specs,
                out_specs=out_specs,
                check_rep=False,
            ),
            donate_argnums=donate,
            keep_unused=True,
        )

    def run(self, concat_inputs):
        """concat_inputs: dict name -> (8*shape0, ...) array."""
        args = [concat_inputs[n] for n in self.in_names]
        zeros = [
            np.zeros((NCORES * z.shape[0], *z.shape[1:]), z.dtype)
            for z in self.zero_outs
        ]
        outs = self.sharded(*args, *zeros)
        return {n: np.asarray(outs[i]) for i, n in enumerate(self.out_names)}


def _get_runner():
    global _RUNNER
    if _RUNNER is None:
        _RUNNER = _Runner()
    return _RUNNER


def pack_inputs(feature, weight):
    """Host-side packing: returns dict of concatenated per-core inputs."""
    feature = np.ascontiguousarray(np.asarray(feature, dtype=np.float32))
    weight = np.ascontiguousarray(np.asarray(weight, dtype=np.float32))
    # featT pack: fpk[core][p*64+e, bb*F+i] = feature[core*BLOC + 2*bb + p, i, e]
    ft = feature.reshape(NCORES, NPAIR, 2, F, E)  # [core, bb, p, i, e]
    fpk = np.ascontiguousarray(ft.transpose(0, 2, 4, 1, 3)).reshape(
        NCORES * 128, NPAIR * F
    )
    wt = np.ascontiguousarray(weight.transpose(1, 0, 2)).reshape(E, F * E)
    wpk_one = np.concatenate([wt, wt], axis=0)  # (128, F*E)
    wpk = np.tile(wpk_one, (NCORES, 1))
    return {"fpk": fpk, "wpk": wpk}


def kernel(feature, weight):
    r = _get_runner()
    ins = pack_inputs(feature, weight)
    outs = r.run(ins)
    return unpack_output(outs["out"])


def unpack_output(out_dev):
    """out_dev: (8*100, BLOC, 2, F) device layout -> (B, F, F)."""
    o = out_dev.reshape(NCORES, 100, BLOC, 2, F)
    # out[core, b, 2p+ci, j] = o[core, p, b, ci, j]
    return np.ascontiguousarray(o.transpose(0, 2, 1, 3, 4)).reshape(B, F, F)


if __name__ == "__main__":
    rng = np.random.default_rng(0)
    feature = rng.standard_normal((B, F, E), dtype=np.float32)
    weight = (0.01 * rng.standard_normal((F, E, E))).astype(np.float32)
    got = kernel(feature, weight)
    bil = np.einsum("bie,ied->bid", feature.astype(np.float64), weight.astype(np.float64))
    ref = np.einsum("bid,bjd->bij", bil, feature.astype(np.float64))
    err = np.abs(got - ref)
    denom = np.abs(ref).max()
    print("max abs err:", err.max(), "rel(scale):", err.max() / denom)
    l2 = np.linalg.norm((got - ref).ravel()) / np.linalg.norm(ref.ravel())
    print("L2 rel:", l2)
